# revision 1
# baseline (speedup 1.0000x reference)
"""Trainium2 Bass kernel for the NeuroSAT-style message-passing core.

Math (see reference):
  M_l    = (W_Lmsg @ L_t + b_Lmsg) @ A_t                      [B,64,NC]
  C_new  = lrelu(W_Cu @ [C_t; M_l] + b_Cu)                    [B,64,NC]
  M_c    = (W_Cmsg @ C_new + b_Cmsg) @ A                      [B,64,NL]
  L_new  = lrelu(W_Lu @ [L_t; M_c; flip(L_t)] + b_Lu)         [B,64,NL]
  U_new  = lrelu(W_Uu @ [sum(L_new); sum(C_new); U_t] + b_Uu) [B,64,1]

Sharding over 8 cores:
  - clauses (NC=16384) sharded 8x2048: each core computes M_l / C_new / CM^T
    for its clause range (reads its A_t column-shard once).
  - CM^T = (W_Cmsg @ C_new)^T shards are AllGathered (fp16, 1MB/core -> 8MB).
  - literals (NL=8192) sharded 8x1024: each core computes M_c / L_new for its
    literal range (reads its A column-shard once).
  - tiny AllReduce of per-core [sum(L_new); sum(C_new)] partials for U_new.

The two giant adjacency matmuls run in fp16 (A is 0/1 -> exact; the message
operands round to fp16; fp32 PSUM accumulation). Everything else is fp32.
"""

import sys

if "/opt/trn_rl_repo" not in sys.path:
    sys.path.insert(0, "/opt/trn_rl_repo")

import numpy as np

from concourse import bacc, bass, mybir, tile
from concourse import bass_utils

B = 4
D = 64
NL = 8192
NC = 16384
NCORES = 8
CSH = NC // NCORES  # 2048 clauses per core
LSH = NL // NCORES  # 1024 literals per core
SLOPE = 0.01

F32 = mybir.dt.float32
F16 = mybir.dt.float16
AX = mybir.AxisListType.X
ADD = mybir.AluOpType.add
LRELU = mybir.ActivationFunctionType.Lrelu

_CACHE = {}


def _build(msg_bias: bool):
    """Build + compile the SPMD Bass program. msg_bias enables the general
    path for nonzero b_Lmsg/b_Cmsg (extra adjacency-colsum matmuls)."""
    nc = bacc.Bacc("TRN2", target_bir_lowering=False, debug=False,
                   num_devices=NCORES)

    din = {}
    def inp(name, shape, dtype):
        din[name] = nc.dram_tensor(name, list(shape), dtype,
                                   kind="ExternalInput").ap()
        return din[name]

    # per-core inputs (see _prep_inputs for layouts)
    lpair = inp("lpair", [2, 128, NL], F16)          # [bp][b0;b1 x dl][l]
    w2lmsg = inp("w2lmsg", [128, 128], F16)          # kron(I2, W_Lmsg.T)
    at_t = inp("at_t", [2, 16, 128, 4096], F16)      # A_t col-shard tiles
    a_t = inp("a_t", [32, 128, 4096], F16)           # A col-shard tiles
    c_shard = inp("c_shard", [B, D, CSH], F32)
    l_shard = inp("l_shard", [B, D, LSH], F32)
    l_flip = inp("l_flip", [B, D, LSH], F32)
    wcut_ev = inp("wcut_ev", [128, D], F32)          # [M_l;C_t] weight rows
    wcut_od = inp("wcut_od", [128, D], F32)          # [C_t;M_l] weight rows
    wcmsgt = inp("wcmsgt", [D, D], F32)
    wlut_a_ev = inp("wlut_a_ev", [128, D], F32)      # [M_c;L_t] rows
    wlut_a_od = inp("wlut_a_od", [128, D], F32)      # [L_t;M_c] rows
    wlut_b = inp("wlut_b", [D, D], F32)              # flip(L_t) rows
    wuut_a = inp("wuut_a", [128, D], F32)            # [sumL;sumC] rows
    wuut_b = inp("wuut_b", [D, D], F32)              # U_t rows
    b_cu = inp("b_cu", [D, 1], F32)
    b_lu = inp("b_lu", [D, 1], F32)
    b_uu = inp("b_uu", [D, 1], F32)
    u_t = inp("u_t", [D, B], F32)
    if msg_bias:
        ones128 = inp("ones128", [128, 1], F16)
        b_lmsg_row = inp("b_lmsg_row", [1, D], F32)
        b_cmsg_row = inp("b_cmsg_row", [1, D], F32)

    c_out = nc.dram_tensor("c_out", [B, D, CSH], F32, kind="ExternalOutput").ap()
    l_out = nc.dram_tensor("l_out", [B, D, LSH], F32, kind="ExternalOutput").ap()
    u_out = nc.dram_tensor("u_out", [D, B], F32, kind="ExternalOutput").ap()

    with tile.TileContext(nc) as tc:
        with tc.tile_pool(name="persist", bufs=1) as pp, \
             tc.tile_pool(name="dram", bufs=1, space="DRAM") as dp:
            # ---- persistent SBUF: weights, accumulators, LM^T ----
            wcut_ev_sb = pp.tile([128, D], F32, name="wcut_ev_sb")
            wcut_od_sb = pp.tile([128, D], F32, name="wcut_od_sb")
            wcmsgt_sb = pp.tile([D, D], F32, name="wcmsgt_sb")
            wlut_a_ev_sb = pp.tile([128, D], F32, name="wlut_a_ev_sb")
            wlut_a_od_sb = pp.tile([128, D], F32, name="wlut_a_od_sb")
            wlut_b_sb = pp.tile([D, D], F32, name="wlut_b_sb")
            wuut_a_sb = pp.tile([128, D], F32, name="wuut_a_sb")
            wuut_b_sb = pp.tile([D, D], F32, name="wuut_b_sb")
            b_cu_sb = pp.tile([D, 1], F32, name="b_cu_sb")
            b_lu_sb = pp.tile([D, 1], F32, name="b_lu_sb")
            b_uu_sb = pp.tile([D, 1], F32, name="b_uu_sb")
            u_t_sb = pp.tile([D, B], F32, name="u_t_sb")
            for sb, src in [(wcut_ev_sb, wcut_ev), (wcut_od_sb, wcut_od),
                            (wcmsgt_sb, wcmsgt), (wlut_a_ev_sb, wlut_a_ev),
                            (wlut_a_od_sb, wlut_a_od), (wlut_b_sb, wlut_b),
                            (wuut_a_sb, wuut_a), (wuut_b_sb, wuut_b),
                            (b_cu_sb, b_cu), (b_lu_sb, b_lu),
                            (b_uu_sb, b_uu), (u_t_sb, u_t)]:
                nc.gpsimd.dma_start(sb[:], src[:])
            if msg_bias:
                ones_sb = pp.tile([128, 1], F16, name="ones_sb")
                blm_sb = pp.tile([1, D], F32, name="blm_sb")
                bcm_sb = pp.tile([1, D], F32, name="bcm_sb")
                nc.gpsimd.dma_start(ones_sb[:], ones128[:])
                nc.gpsimd.dma_start(blm_sb[:], b_lmsg_row[:])
                nc.gpsimd.dma_start(bcm_sb[:], b_cmsg_row[:])

            csum = [pp.tile([D, 1], F32, name=f"csum{b}") for b in range(B)]
            lsum = [pp.tile([D, 1], F32, name=f"lsum{b}") for b in range(B)]

            # LM^T [l, m] fp16; col m = bp*128 + b_local*64 + dc
            lmt = pp.tile([128, 64 * 256], F16, name="lmt")

            # collective bounce buffers
            cc_in = dp.tile([CSH, 256], F16, name="cc_in")
            cc_out = dp.tile([128, 128, 256], F16, name="cc_out",
                             addr_space="Shared")
            ucc_in = dp.tile([128, B], F32, name="ucc_in")
            ucc_out = dp.tile([128, B], F32, name="ucc_out",
                              addr_space="Shared")

            # ---- phase A: LM^T = (W_Lmsg @ L_t)^T in fp16 ----
            with tc.tile_pool(name="pha", bufs=1) as pa, \
                 tc.tile_pool(name="pha_ps", bufs=4, space="PSUM") as pa_ps:
                w2_sb = pa.tile([128, 128], F16, name="w2_sb")
                nc.gpsimd.dma_start(w2_sb[:], w2lmsg[:])
                lp_sb = pa.tile([128, 2 * NL], F16, name="lp_sb")
                nc.sync.dma_start(lp_sb[:, 0:NL], lpair[0])
                nc.sync.dma_start(lp_sb[:, NL:2 * NL], lpair[1])
                for lt in range(64):
                    for bp in range(2):
                        ps = pa_ps.tile([128, 128], F32, name="lmt_ps")
                        nc.tensor.matmul(
                            ps[:],
                            lp_sb[:, bp * NL + lt * 128: bp * NL + (lt + 1) * 128],
                            w2_sb[:], start=True, stop=True)
                        nc.vector.tensor_copy(
                            lmt[:, lt * 256 + bp * 128: lt * 256 + (bp + 1) * 128],
                            ps[:])

            # ---- phase B: M_l -> C_new -> CM^T over clause shard ----
            with tc.tile_pool(name="phb", bufs=3) as pb, \
                 tc.tile_pool(name="phb_cn", bufs=5) as pb_cn, \
                 tc.tile_pool(name="phb_ml", bufs=2, space="PSUM") as pb_ml, \
                 tc.tile_pool(name="phb_cu", bufs=2, space="PSUM") as pb_cu, \
                 tc.tile_pool(name="phb_cm", bufs=2, space="PSUM") as pb_cm:
                for ct in range(2):
                    ml_ps = [pb_ml.tile([128, 1024], F32, name="ml_ps")
                             for _ in range(2)]
                    if msg_bias:
                        s_ps = pb_cm.tile([1, 1024], F32, name="s_ps")
                    for k4 in range(16):
                        at4 = pb.tile([128, 4096], F16, name="at4")
                        nc.sync.dma_start(at4[:], at_t[ct, k4])
                        for g in range(4):
                            kc = k4 * 4 + g
                            st, sp = kc == 0, kc == 63
                            for bp in range(2):
                                for h in range(2):
                                    nc.tensor.matmul(
                                        ml_ps[bp][:, h * 512:(h + 1) * 512],
                                        lmt[:, kc * 256 + bp * 128:
                                            kc * 256 + (bp + 1) * 128],
                                        at4[:, g * 1024 + h * 512:
                                            g * 1024 + (h + 1) * 512],
                                        start=st, stop=sp)
                            if msg_bias:
                                for h in range(2):
                                    nc.tensor.matmul(
                                        s_ps[:, h * 512:(h + 1) * 512],
                                        ones_sb[:],
                                        at4[:, g * 1024 + h * 512:
                                            g * 1024 + (h + 1) * 512],
                                        start=st, stop=sp)
                    if msg_bias:
                        # M_l bias term: b_Lmsg (x) colsum(A_t)
                        s_sb = pb.tile([1, 1024], F32, name="s_sb")
                        nc.vector.tensor_copy(s_sb[:], s_ps[:])
                        mlb_ps = pb_cu.tile([128, 1024], F32, name="mlb_ps")
                        for h in range(2):
                            hs = slice(h * 512, (h + 1) * 512)
                            nc.tensor.matmul(mlb_ps[0:64, hs], blm_sb[:],
                                             s_sb[:, hs], start=True, stop=True)
                            nc.tensor.matmul(mlb_ps[64:128, hs], blm_sb[:],
                                             s_sb[:, hs], start=True, stop=True)

                    cn_tiles = []
                    for b in range(B):
                        bp, odd = b >> 1, b & 1
                        ccat = pb.tile([128, 1024], F32, name="ccat")
                        # even b: [M_l; C_t]; odd b: [C_t; M_l] (no partition shift)
                        crows = slice(64, 128) if not odd else slice(0, 64)
                        mrows = slice(0, 64) if not odd else slice(64, 128)
                        nc.gpsimd.dma_start(
                            ccat[crows, :],
                            c_shard[b, :, ct * 1024:(ct + 1) * 1024])
                        nc.vector.tensor_copy(ccat[mrows, :],
                                              ml_ps[bp][mrows, :])
                        if msg_bias:
                            nc.vector.tensor_add(ccat[mrows, :], ccat[mrows, :],
                                                 mlb_ps[mrows, :])
                        wc = wcut_od_sb if odd else wcut_ev_sb
                        cn = pb_cn.tile([D, 1024], F32, name="cn")
                        for h in range(2):
                            hs = slice(h * 512, (h + 1) * 512)
                            cu_ps = pb_cu.tile([D, 512], F32, name="cu_ps")
                            nc.tensor.matmul(cu_ps[:], wc[:], ccat[:, hs],
                                             start=True, stop=True)
                            nc.scalar.activation(cn[:, hs], cu_ps[:], LRELU,
                                                 bias=b_cu_sb[:], alpha=SLOPE)
                        nc.gpsimd.dma_start(
                            c_out[b, :, ct * 1024:(ct + 1) * 1024], cn[:])
                        red = pb.tile([D, 1], F32, name="credt")
                        nc.vector.tensor_reduce(red[:], cn[:], axis=AX, op=ADD)
                        if ct == 0:
                            nc.vector.tensor_copy(csum[b][:], red[:])
                        else:
                            nc.vector.tensor_add(csum[b][:], csum[b][:], red[:])
                        cn_tiles.append(cn)

                    # CM^T blocks -> cc_in
                    for cc in range(8):
                        cmt_sb = pb.tile([128, 256], F16, name="cmt_sb")
                        for b in range(B):
                            cmt_ps = pb_cm.tile([128, D], F32, name="cmt_ps")
                            nc.tensor.matmul(
                                cmt_ps[:],
                                cn_tiles[b][:, cc * 128:(cc + 1) * 128],
                                wcmsgt_sb[:], start=True, stop=True)
                            off = (b >> 1) * 128 + (b & 1) * 64
                            nc.vector.tensor_copy(cmt_sb[:, off:off + 64],
                                                  cmt_ps[:])
                        nc.gpsimd.dma_start(
                            cc_in[ct * 1024 + cc * 128:
                                  ct * 1024 + (cc + 1) * 128, :],
                            cmt_sb[:])

            nc.gpsimd.collective_compute(
                "AllGather", mybir.AluOpType.bypass,
                replica_groups=[list(range(NCORES))],
                ins=[cc_in.opt()], outs=[cc_out.opt()])

            # ---- phase C: M_c over literal shard ----
            with tc.tile_pool(name="phc", bufs=3) as pc, \
                 tc.tile_pool(name="phc_cg", bufs=6) as pc_cg, \
                 tc.tile_pool(name="phc_ps", bufs=2, space="PSUM") as pc_ps:
                mc_ps = [pc_ps.tile([128, 1024], F32, name="mc_ps")
                         for _ in range(2)]
                if msg_bias:
                    s2_ps = pc_ps.tile([1, 1024], F32, name="s2_ps", bufs=1)
                for k4 in range(32):
                    a4 = pc.tile([128, 4096], F16, name="a4")
                    nc.sync.dma_start(a4[:], a_t[k4])
                    for g in range(4):
                        kc = k4 * 4 + g
                        st, sp = kc == 0, kc == 127
                        cg = pc_cg.tile([128, 256], F16, name="cg")
                        nc.gpsimd.dma_start(cg[:], cc_out[kc])
                        for bp in range(2):
                            for h in range(2):
                                nc.tensor.matmul(
                                    mc_ps[bp][:, h * 512:(h + 1) * 512],
                                    cg[:, bp * 128:(bp + 1) * 128],
                                    a4[:, g * 1024 + h * 512:
                                       g * 1024 + (h + 1) * 512],
                                    start=st, stop=sp)
                        if msg_bias:
                            for h in range(2):
                                nc.tensor.matmul(
                                    s2_ps[:, h * 512:(h + 1) * 512],
                                    ones_sb[:],
                                    a4[:, g * 1024 + h * 512:
                                       g * 1024 + (h + 1) * 512],
                                    start=st, stop=sp)

                # ---- phase D: L update ----
                if msg_bias:
                    s2_sb = pc.tile([1, 1024], F32, name="s2_sb")
                    nc.vector.tensor_copy(s2_sb[:], s2_ps[:])
                    mcb_ps = pc_ps.tile([128, 1024], F32, name="mcb_ps", bufs=1)
                    for h in range(2):
                        hs = slice(h * 512, (h + 1) * 512)
                        nc.tensor.matmul(mcb_ps[0:64, hs], bcm_sb[:],
                                         s2_sb[:, hs], start=True, stop=True)
                        nc.tensor.matmul(mcb_ps[64:128, hs], bcm_sb[:],
                                         s2_sb[:, hs], start=True, stop=True)
                with tc.tile_pool(name="phd", bufs=3) as pd, \
                     tc.tile_pool(name="phd_ps", bufs=2, space="PSUM") as pd_ps:
                    for b in range(B):
                        bp, odd = b >> 1, b & 1
                        lcat = pd.tile([128, 1024], F32, name="lcat")
                        lrows = slice(64, 128) if not odd else slice(0, 64)
                        mrows = slice(0, 64) if not odd else slice(64, 128)
                        nc.gpsimd.dma_start(lcat[lrows, :], l_shard[b])
                        nc.vector.tensor_copy(lcat[mrows, :], mc_ps[bp][mrows, :])
                        if msg_bias:
                            nc.vector.tensor_add(lcat[mrows, :], lcat[mrows, :],
                                                 mcb_ps[mrows, :])
                        lf = pd.tile([D, 1024], F32, name="lf")
                        nc.gpsimd.dma_start(lf[:], l_flip[b])
                        wl = wlut_a_od_sb if odd else wlut_a_ev_sb
                        ln = pd.tile([D, 1024], F32, name="ln")
                        for h in range(2):
                            hs = slice(h * 512, (h + 1) * 512)
                            lu_ps = pd_ps.tile([D, 512], F32, name="lu_ps")
                            nc.tensor.matmul(lu_ps[:], wl[:], lcat[:, hs],
                                             start=True, stop=False)
                            nc.tensor.matmul(lu_ps[:], wlut_b_sb[:], lf[:, hs],
                                             start=False, stop=True)
                            nc.scalar.activation(ln[:, hs], lu_ps[:], LRELU,
                                                 bias=b_lu_sb[:], alpha=SLOPE)
                        nc.gpsimd.dma_start(l_out[b], ln[:])
                        red = pd.tile([D, 1], F32, name="lredt")
                        nc.vector.tensor_reduce(red[:], ln[:], axis=AX, op=ADD)
                        nc.vector.tensor_copy(lsum[b][:], red[:])

                    # ---- phase E: U update ----
                    upack = pd.tile([128, B], F32, name="upack")
                    for b in range(B):
                        nc.gpsimd.dma_start(upack[0:64, b:b + 1], lsum[b][:])
                        nc.gpsimd.dma_start(upack[64:128, b:b + 1], csum[b][:])
                    nc.gpsimd.dma_start(ucc_in[:], upack[:])
                    nc.gpsimd.collective_compute(
                        "AllReduce", ADD,
                        replica_groups=[list(range(NCORES))],
                        ins=[ucc_in.opt()], outs=[ucc_out.opt()])
                    usums = pd.tile([128, B], F32, name="usums")
                    nc.gpsimd.dma_start(usums[:], ucc_out[:])
                    u_ps = pd_ps.tile([D, B], F32, name="u_ps")
                    nc.tensor.matmul(u_ps[:], wuut_a_sb[:], usums[:],
                                     start=True, stop=False)
                    nc.tensor.matmul(u_ps[:], wuut_b_sb[:], u_t_sb[:],
                                     start=False, stop=True)
                    un = pd.tile([D, B], F32, name="un")
                    nc.scalar.activation(un[:], u_ps[:], LRELU,
                                         bias=b_uu_sb[:], alpha=SLOPE)
                    nc.gpsimd.dma_start(u_out[:], un[:])

    nc.compile()
    return nc


def _prep_inputs(L_t, C_t, U_t, A, A_t,
                 W_Lmsg, b_Lmsg, W_Cmsg, b_Cmsg,
                 W_Lu, b_Lu, W_Cu, b_Cu, W_Uu, b_Uu, msg_bias):
    f16, f32 = np.float16, np.float32

    # replicated tensors
    lp = np.stack([
        np.concatenate([L_t[0], L_t[1]], axis=0),
        np.concatenate([L_t[2], L_t[3]], axis=0),
    ]).astype(f16)                                           # [2,128,NL]
    w2 = np.kron(np.eye(2, dtype=f32), W_Lmsg.T).astype(f16)  # [128,128]
    wcut_od = W_Cu.T.astype(f32)
    wcut_ev = np.concatenate([W_Cu.T[64:128], W_Cu.T[0:64]]).astype(f32)
    wlt = W_Lu.T.astype(f32)
    wlut_a_od = wlt[0:128].copy()
    wlut_a_ev = np.concatenate([wlt[64:128], wlt[0:64]]).astype(f32)
    wlut_b = wlt[128:192].copy()
    wut = W_Uu.T.astype(f32)
    wuut_a = wut[0:128].copy()
    wuut_b = wut[128:192].copy()
    rep = dict(
        lpair=lp, w2lmsg=w2,
        wcut_ev=wcut_ev, wcut_od=wcut_od, wcmsgt=W_Cmsg.T.astype(f32).copy(),
        wlut_a_ev=wlut_a_ev, wlut_a_od=wlut_a_od, wlut_b=wlut_b,
        wuut_a=wuut_a, wuut_b=wuut_b,
        b_cu=b_Cu.reshape(D, 1).astype(f32),
        b_lu=b_Lu.reshape(D, 1).astype(f32),
        b_uu=b_Uu.reshape(D, 1).astype(f32),
        u_t=U_t.reshape(B, D).T.astype(f32).copy(),
    )
    if msg_bias:
        rep["ones128"] = np.ones((128, 1), f16)
        rep["b_lmsg_row"] = b_Lmsg.reshape(1, D).astype(f32)
        rep["b_cmsg_row"] = b_Cmsg.reshape(1, D).astype(f32)

    A16 = A.astype(f16)
    At16 = A_t.astype(f16)
    in_maps = []
    for r in range(NCORES):
        atr = At16[:, r * CSH:(r + 1) * CSH]                 # [8192, 2048]
        at_tiles = (atr.reshape(16, 4, 128, 2, 1024)
                    .transpose(3, 0, 2, 1, 4).reshape(2, 16, 128, 4096))
        ar = A16[:, r * LSH:(r + 1) * LSH]                   # [16384, 1024]
        a_tiles = (ar.reshape(32, 4, 128, 1024)
                   .transpose(0, 2, 1, 3).reshape(32, 128, 4096))
        fl = ((r + NCORES // 2) % NCORES) * LSH
        m = dict(rep)
        m.update(
            at_t=np.ascontiguousarray(at_tiles),
            a_t=np.ascontiguousarray(a_tiles),
            c_shard=np.ascontiguousarray(C_t[:, :, r * CSH:(r + 1) * CSH]),
            l_shard=np.ascontiguousarray(L_t[:, :, r * LSH:(r + 1) * LSH]),
            l_flip=np.ascontiguousarray(L_t[:, :, fl:fl + LSH]),
        )
        in_maps.append(m)
    return in_maps


def kernel(L_t, C_t, U_t, A, A_t,
           W_Lmsg, b_Lmsg, W_Cmsg, b_Cmsg,
           W_Lu, b_Lu, W_Cu, b_Cu, W_Uu, b_Uu, _trace=False):
    args = [np.asarray(x, dtype=np.float32) for x in
            (L_t, C_t, U_t, A, A_t, W_Lmsg, b_Lmsg, W_Cmsg, b_Cmsg,
             W_Lu, b_Lu, W_Cu, b_Cu, W_Uu, b_Uu)]
    (L_t, C_t, U_t, A, A_t, W_Lmsg, b_Lmsg, W_Cmsg, b_Cmsg,
     W_Lu, b_Lu, W_Cu, b_Cu, W_Uu, b_Uu) = args

    msg_bias = bool(np.any(b_Lmsg) or np.any(b_Cmsg))
    if msg_bias not in _CACHE:
        _CACHE[msg_bias] = _build(msg_bias)
    nc = _CACHE[msg_bias]

    in_maps = _prep_inputs(L_t, C_t, U_t, A, A_t, W_Lmsg, b_Lmsg,
                           W_Cmsg, b_Cmsg, W_Lu, b_Lu, W_Cu, b_Cu,
                           W_Uu, b_Uu, msg_bias)
    res = bass_utils.run_bass_kernel_spmd(
        nc, in_maps, core_ids=list(range(NCORES)), trace=_trace)

    C_new = np.concatenate([res.results[r]["c_out"] for r in range(NCORES)],
                           axis=2)
    L_new = np.concatenate([res.results[r]["l_out"] for r in range(NCORES)],
                           axis=2)
    U_new = res.results[0]["u_out"].T.reshape(B, D, 1).astype(np.float32)
    out = (L_new.astype(np.float32), C_new.astype(np.float32), U_new)
    if _trace:
        return out, res
    return out


# revision 3
# speedup vs baseline: 1.1292x; 1.1292x over previous
"""Trainium2 Bass kernel for the NeuroSAT-style message-passing core.

Math (see reference):
  M_l    = (W_Lmsg @ L_t + b_Lmsg) @ A_t                      [B,64,NC]
  C_new  = lrelu(W_Cu @ [C_t; M_l] + b_Cu)                    [B,64,NC]
  M_c    = (W_Cmsg @ C_new + b_Cmsg) @ A                      [B,64,NL]
  L_new  = lrelu(W_Lu @ [L_t; M_c; flip(L_t)] + b_Lu)         [B,64,NL]
  U_new  = lrelu(W_Uu @ [sum(L_new); sum(C_new); U_t] + b_Uu) [B,64,1]

Sharding over 8 cores:
  - clauses (NC=16384) sharded 8x2048: each core computes M_l / C_new / CM^T
    for its clause range (reads its A_t column-shard once).
  - CM^T = (W_Cmsg @ C_new)^T shards are AllGathered in two halves (fp16,
    0.5MB/core each) so the gather overlaps compute on both sides.
  - literals (NL=8192) sharded 8x1024: each core computes M_c / L_new for its
    literal range (reads its A column-shard once, rows permuted to match the
    half-AllGather arrival order).
  - per-core [sum(L_new); sum(C_new)] partials are AllGathered (2KB) and
    summed locally for the U update.

The adjacency matmuls and per-node linears run in fp16 (A is 0/1 -> exact;
fp32 PSUM accumulation); biases/activations/outputs are fp32.
"""

import sys

if "/opt/trn_rl_repo" not in sys.path:
    sys.path.insert(0, "/opt/trn_rl_repo")

import numpy as np

from concourse import bacc, bass, mybir, tile
from concourse import bass_utils

B = 4
D = 64
NL = 8192
NC = 16384
NCORES = 8
CSH = NC // NCORES  # 2048 clauses per core
LSH = NL // NCORES  # 1024 literals per core
SLOPE = 0.01

F32 = mybir.dt.float32
F16 = mybir.dt.float16
AX = mybir.AxisListType.X
ADD = mybir.AluOpType.add
LRELU = mybir.ActivationFunctionType.Lrelu

_CACHE = {}


def _build(msg_bias: bool):
    nc = bacc.Bacc("TRN2", target_bir_lowering=False, debug=False,
                   num_devices=NCORES)

    def inp(name, shape, dtype):
        return nc.dram_tensor(name, list(shape), dtype,
                              kind="ExternalInput").ap()

    lpair = inp("lpair", [2, 128, NL], F16)          # [bp][b0;b1 x dl][l]
    w2lmsg = inp("w2lmsg", [128, 128], F16)          # kron(I2, W_Lmsg.T)
    at_t = inp("at_t", [2, 16, 128, 4096], F16)      # A_t col-shard tiles
    a_t = inp("a_t", [32, 128, 4096], F16)           # A col-shard tiles (perm)
    c_shard = inp("c_shard", [B, D, CSH], F16)
    l_shard = inp("l_shard", [B, D, LSH], F16)
    l_flip = inp("l_flip", [B, D, LSH], F16)
    wcut_ev = inp("wcut_ev", [128, D], F16)          # [M_l;C_t] weight rows
    wcut_od = inp("wcut_od", [128, D], F16)          # [C_t;M_l] weight rows
    wcmsgt = inp("wcmsgt", [D, D], F16)
    wlut_a_ev = inp("wlut_a_ev", [128, D], F16)      # [M_c;L_t] rows
    wlut_a_od = inp("wlut_a_od", [128, D], F16)      # [L_t;M_c] rows
    wlut_b = inp("wlut_b", [D, D], F16)              # flip(L_t) rows
    wuut_a = inp("wuut_a", [128, D], F32)            # [sumL;sumC] rows
    wuut_b = inp("wuut_b", [D, D], F32)              # U_t rows
    b_cu = inp("b_cu", [D, 1], F32)
    b_lu = inp("b_lu", [D, 1], F32)
    b_uu = inp("b_uu", [D, 1], F32)
    u_t = inp("u_t", [D, B], F32)
    if msg_bias:
        ones128 = inp("ones128", [128, 1], F16)
        b_lmsg_row = inp("b_lmsg_row", [1, D], F32)
        b_cmsg_row = inp("b_cmsg_row", [1, D], F32)

    c_out = nc.dram_tensor("c_out", [B, D, CSH], F32, kind="ExternalOutput").ap()
    l_out = nc.dram_tensor("l_out", [B, D, LSH], F32, kind="ExternalOutput").ap()
    u_out = nc.dram_tensor("u_out", [D, B], F32, kind="ExternalOutput").ap()

    with tile.TileContext(nc) as tc:
        with tc.tile_pool(name="persist", bufs=1) as pp, \
             tc.tile_pool(name="dram", bufs=1, space="DRAM") as dp:
            # ---- persistent SBUF: weights, accumulators, LM^T ----
            wcut_ev_sb = pp.tile([128, D], F16, name="wcut_ev_sb")
            wcut_od_sb = pp.tile([128, D], F16, name="wcut_od_sb")
            wcmsgt_sb = pp.tile([D, D], F16, name="wcmsgt_sb")
            wlut_a_ev_sb = pp.tile([128, D], F16, name="wlut_a_ev_sb")
            wlut_a_od_sb = pp.tile([128, D], F16, name="wlut_a_od_sb")
            wlut_b_sb = pp.tile([D, D], F16, name="wlut_b_sb")
            wuut_a_sb = pp.tile([128, D], F32, name="wuut_a_sb")
            wuut_b_sb = pp.tile([D, D], F32, name="wuut_b_sb")
            b_cu_sb = pp.tile([D, 1], F32, name="b_cu_sb")
            b_lu_sb = pp.tile([D, 1], F32, name="b_lu_sb")
            b_uu_sb = pp.tile([D, 1], F32, name="b_uu_sb")
            u_t_sb = pp.tile([D, B], F32, name="u_t_sb")
            for sb, src in [(wcut_ev_sb, wcut_ev), (wcut_od_sb, wcut_od),
                            (wcmsgt_sb, wcmsgt), (wlut_a_ev_sb, wlut_a_ev),
                            (wlut_a_od_sb, wlut_a_od), (wlut_b_sb, wlut_b),
                            (wuut_a_sb, wuut_a), (wuut_b_sb, wuut_b),
                            (b_cu_sb, b_cu), (b_lu_sb, b_lu),
                            (b_uu_sb, b_uu), (u_t_sb, u_t)]:
                nc.gpsimd.dma_start(sb[:], src[:])
            if msg_bias:
                ones_sb = pp.tile([128, 1], F16, name="ones_sb")
                blm_sb = pp.tile([1, D], F32, name="blm_sb")
                bcm_sb = pp.tile([1, D], F32, name="bcm_sb")
                nc.gpsimd.dma_start(ones_sb[:], ones128[:])
                nc.gpsimd.dma_start(blm_sb[:], b_lmsg_row[:])
                nc.gpsimd.dma_start(bcm_sb[:], b_cmsg_row[:])

            csum = [pp.tile([D, 1], F32, name=f"csum{b}") for b in range(B)]

            # LM^T [l, m] fp16; col m = bp*128 + b_local*64 + dc
            lmt = pp.tile([128, 64 * 256], F16, name="lmt")

            # collective bounce buffers (AllGather in two clause halves)
            cc_in = [dp.tile([64 * 128 // 8, 256], F16, name=f"cc_in{h}")
                     for h in range(2)]
            cc_out = [dp.tile([64, 128, 256], F16, name=f"cc_out{h}",
                              addr_space="Shared") for h in range(2)]
            ucc_in = dp.tile([128, B], F32, name="ucc_in")
            ucc_out = dp.tile([NCORES, 128, B], F32, name="ucc_out",
                              addr_space="Shared")

            # ---- phase A: LM^T = (W_Lmsg @ L_t)^T in fp16 ----
            with tc.tile_pool(name="pha", bufs=1) as pa, \
                 tc.tile_pool(name="pha_ps", bufs=4, space="PSUM") as pa_ps:
                w2_sb = pa.tile([128, 128], F16, name="w2_sb")
                nc.sync.dma_start(w2_sb[:], w2lmsg[:])
                lp_sb = pa.tile([128, 2 * NL], F16, name="lp_sb")
                for ch in range(4):
                    nc.sync.dma_start(
                        lp_sb[:, ch * 4096:(ch + 1) * 4096],
                        lpair[ch // 2, :, (ch % 2) * 4096:(ch % 2 + 1) * 4096])
                for lt in range(64):
                    for bp in range(2):
                        ps = pa_ps.tile([128, 128], F32, name="lmt_ps")
                        nc.tensor.matmul(
                            ps[:],
                            lp_sb[:, bp * NL + lt * 128: bp * NL + (lt + 1) * 128],
                            w2_sb[:], start=True, stop=True)
                        nc.vector.tensor_copy(
                            lmt[:, lt * 256 + bp * 128: lt * 256 + (bp + 1) * 128],
                            ps[:])

            # ---- phase B: M_l -> C_new -> CM^T over clause shard ----
            with tc.tile_pool(name="phb", bufs=4) as pb, \
                 tc.tile_pool(name="phb_cn", bufs=5) as pb_cn, \
                 tc.tile_pool(name="phb_ml", bufs=2, space="PSUM") as pb_ml, \
                 tc.tile_pool(name="phb_cu", bufs=2, space="PSUM") as pb_cu, \
                 tc.tile_pool(name="phb_cm", bufs=2, space="PSUM") as pb_cm:
                for ct in range(2):
                    ml_ps = [pb_ml.tile([128, 1024], F32, name="ml_ps")
                             for _ in range(2)]
                    if msg_bias:
                        s_ps = pb_cm.tile([1, 1024], F32, name="s_ps")
                    for k4 in range(16):
                        at4 = pb.tile([128, 4096], F16, name="at4")
                        nc.sync.dma_start(at4[:], at_t[ct, k4])
                        for g in range(4):
                            kc = k4 * 4 + g
                            st, sp = kc == 0, kc == 63
                            for bp in range(2):
                                for h in range(2):
                                    nc.tensor.matmul(
                                        ml_ps[bp][:, h * 512:(h + 1) * 512],
                                        lmt[:, kc * 256 + bp * 128:
                                            kc * 256 + (bp + 1) * 128],
                                        at4[:, g * 1024 + h * 512:
                                            g * 1024 + (h + 1) * 512],
                                        start=st, stop=sp)
                            if msg_bias:
                                for h in range(2):
                                    nc.tensor.matmul(
                                        s_ps[:, h * 512:(h + 1) * 512],
                                        ones_sb[:],
                                        at4[:, g * 1024 + h * 512:
                                            g * 1024 + (h + 1) * 512],
                                        start=st, stop=sp)
                    if msg_bias:
                        s_sb = pb.tile([1, 1024], F32, name="s_sb")
                        nc.vector.tensor_copy(s_sb[:], s_ps[:])
                        mlb_ps = pb_cu.tile([128, 1024], F32, name="mlb_ps")
                        for h in range(2):
                            hs = slice(h * 512, (h + 1) * 512)
                            nc.tensor.matmul(mlb_ps[0:64, hs], blm_sb[:],
                                             s_sb[:, hs], start=True, stop=True)
                            nc.tensor.matmul(mlb_ps[64:128, hs], blm_sb[:],
                                             s_sb[:, hs], start=True, stop=True)

                    cn16_tiles = []
                    for b in range(B):
                        bp, odd = b >> 1, b & 1
                        ccat = pb.tile([128, 1024], F16, name="ccat")
                        # even b: [M_l; C_t]; odd b: [C_t; M_l]
                        crows = slice(64, 128) if not odd else slice(0, 64)
                        mrows = slice(0, 64) if not odd else slice(64, 128)
                        nc.gpsimd.dma_start(
                            ccat[crows, :],
                            c_shard[b, :, ct * 1024:(ct + 1) * 1024])
                        if msg_bias:
                            tmp = pb.tile([64, 1024], F32, name="mltmp")
                            nc.vector.tensor_add(tmp[:], ml_ps[bp][mrows, :],
                                                 mlb_ps[mrows, :])
                            nc.vector.tensor_copy(ccat[mrows, :], tmp[:])
                        else:
                            nc.vector.tensor_copy(ccat[mrows, :],
                                                  ml_ps[bp][mrows, :])
                        wc = wcut_od_sb if odd else wcut_ev_sb
                        cn = pb_cn.tile([D, 1024], F32, name="cn")
                        for h in range(2):
                            hs = slice(h * 512, (h + 1) * 512)
                            cu_ps = pb_cu.tile([D, 512], F32, name="cu_ps")
                            nc.tensor.matmul(cu_ps[:], wc[:], ccat[:, hs],
                                             start=True, stop=True)
                            nc.scalar.activation(cn[:, hs], cu_ps[:], LRELU,
                                                 bias=b_cu_sb[:], alpha=SLOPE)
                        nc.gpsimd.dma_start(
                            c_out[b, :, ct * 1024:(ct + 1) * 1024], cn[:])
                        cn16 = pb_cn.tile([D, 1024], F16, name="cn16")
                        nc.vector.tensor_copy(cn16[:], cn[:])
                        red = pb.tile([D, 1], F32, name="credt")
                        nc.vector.tensor_reduce(red[:], cn[:], axis=AX, op=ADD)
                        if ct == 0:
                            nc.vector.tensor_copy(csum[b][:], red[:])
                        else:
                            nc.vector.tensor_add(csum[b][:], csum[b][:], red[:])
                        cn16_tiles.append(cn16)

                    # CM^T blocks -> cc_in[ct]
                    for cc in range(8):
                        cmt_sb = pb.tile([128, 256], F16, name="cmt_sb")
                        for b in range(B):
                            cmt_ps = pb_cm.tile([128, D], F32, name="cmt_ps")
                            nc.tensor.matmul(
                                cmt_ps[:],
                                cn16_tiles[b][:, cc * 128:(cc + 1) * 128],
                                wcmsgt_sb[:], start=True, stop=True)
                            off = (b >> 1) * 128 + (b & 1) * 64
                            nc.vector.tensor_copy(cmt_sb[:, off:off + 64],
                                                  cmt_ps[:])
                        nc.gpsimd.dma_start(
                            cc_in[ct][cc * 128:(cc + 1) * 128, :], cmt_sb[:])

                    nc.gpsimd.collective_compute(
                        "AllGather", mybir.AluOpType.bypass,
                        replica_groups=[list(range(NCORES))],
                        ins=[cc_in[ct].opt()], outs=[cc_out[ct].opt()])

            # ---- phase C: M_c over literal shard (A rows permuted) ----
            with tc.tile_pool(name="phc", bufs=4) as pc, \
                 tc.tile_pool(name="phc_cg", bufs=4) as pc_cg, \
                 tc.tile_pool(name="phc_ps", bufs=2, space="PSUM") as pc_ps:
                mc_ps = [pc_ps.tile([128, 1024], F32, name="mc_ps")
                         for _ in range(2)]
                if msg_bias:
                    s2_ps = pc_ps.tile([1, 1024], F32, name="s2_ps", bufs=1)
                for hf in range(2):
                    for k4 in range(16):
                        a4 = pc.tile([128, 4096], F16, name="a4")
                        nc.sync.dma_start(a4[:], a_t[hf * 16 + k4])
                        cg4 = pc_cg.tile([128, 1024], F16, name="cg4")
                        nc.scalar.dma_start(
                            cg4[:].rearrange("p (g j) -> p g j", g=4),
                            cc_out[hf][4 * k4:4 * k4 + 4].rearrange(
                                "g p j -> p g j"))
                        for g in range(4):
                            kc = hf * 64 + k4 * 4 + g
                            st, sp = kc == 0, kc == 127
                            for bp in range(2):
                                for h in range(2):
                                    nc.tensor.matmul(
                                        mc_ps[bp][:, h * 512:(h + 1) * 512],
                                        cg4[:, g * 256 + bp * 128:
                                            g * 256 + (bp + 1) * 128],
                                        a4[:, g * 1024 + h * 512:
                                           g * 1024 + (h + 1) * 512],
                                        start=st, stop=sp)
                            if msg_bias:
                                for h in range(2):
                                    nc.tensor.matmul(
                                        s2_ps[:, h * 512:(h + 1) * 512],
                                        ones_sb[:],
                                        a4[:, g * 1024 + h * 512:
                                           g * 1024 + (h + 1) * 512],
                                        start=st, stop=sp)

                # ---- phase D: L update ----
                if msg_bias:
                    s2_sb = pc.tile([1, 1024], F32, name="s2_sb")
                    nc.vector.tensor_copy(s2_sb[:], s2_ps[:])
                    mcb_ps = pc_ps.tile([128, 1024], F32, name="mcb_ps", bufs=1)
                    for h in range(2):
                        hs = slice(h * 512, (h + 1) * 512)
                        nc.tensor.matmul(mcb_ps[0:64, hs], bcm_sb[:],
                                         s2_sb[:, hs], start=True, stop=True)
                        nc.tensor.matmul(mcb_ps[64:128, hs], bcm_sb[:],
                                         s2_sb[:, hs], start=True, stop=True)
                with tc.tile_pool(name="phd", bufs=3) as pd, \
                     tc.tile_pool(name="phd_ps", bufs=2, space="PSUM") as pd_ps:
                    for b in range(B):
                        bp, odd = b >> 1, b & 1
                        lcat = pd.tile([128, 1024], F16, name="lcat")
                        lrows = slice(64, 128) if not odd else slice(0, 64)
                        mrows = slice(0, 64) if not odd else slice(64, 128)
                        nc.gpsimd.dma_start(lcat[lrows, :], l_shard[b])
                        if msg_bias:
                            tmp = pd.tile([64, 1024], F32, name="mctmp")
                            nc.vector.tensor_add(tmp[:], mc_ps[bp][mrows, :],
                                                 mcb_ps[mrows, :])
                            nc.vector.tensor_copy(lcat[mrows, :], tmp[:])
                        else:
                            nc.vector.tensor_copy(lcat[mrows, :],
                                                  mc_ps[bp][mrows, :])
                        lf = pd.tile([D, 1024], F16, name="lf")
                        nc.gpsimd.dma_start(lf[:], l_flip[b])
                        wl = wlut_a_od_sb if odd else wlut_a_ev_sb
                        ln = pd.tile([D, 1024], F32, name="ln")
                        for h in range(2):
                            hs = slice(h * 512, (h + 1) * 512)
                            lu_ps = pd_ps.tile([D, 512], F32, name="lu_ps")
                            nc.tensor.matmul(lu_ps[:], wl[:], lcat[:, hs],
                                             start=True, stop=False)
                            nc.tensor.matmul(lu_ps[:], wlut_b_sb[:], lf[:, hs],
                                             start=False, stop=True)
                            nc.scalar.activation(ln[:, hs], lu_ps[:], LRELU,
                                                 bias=b_lu_sb[:], alpha=SLOPE)
                        nc.gpsimd.dma_start(l_out[b], ln[:])
                        red = pd.tile([D, 1], F32, name="lredt")
                        nc.vector.tensor_reduce(red[:], ln[:], axis=AX, op=ADD)
                        nc.gpsimd.dma_start(ucc_in[0:64, b:b + 1], red[:])
                        nc.gpsimd.dma_start(ucc_in[64:128, b:b + 1], csum[b][:])

                    # ---- phase E: U update (AllGather partials, sum locally) ----
                    nc.gpsimd.collective_compute(
                        "AllGather", mybir.AluOpType.bypass,
                        replica_groups=[list(range(NCORES))],
                        ins=[ucc_in.opt()], outs=[ucc_out.opt()])
                    usums = pd.tile([128, B], F32, name="usums")
                    nc.gpsimd.dma_start(usums[:], ucc_out[0])
                    for r in range(1, NCORES):
                        upart = pd.tile([128, B], F32, name="upart", bufs=4)
                        nc.gpsimd.dma_start(upart[:], ucc_out[r])
                        nc.vector.tensor_add(usums[:], usums[:], upart[:])
                    u_ps = pd_ps.tile([D, B], F32, name="u_ps")
                    nc.tensor.matmul(u_ps[:], wuut_a_sb[:], usums[:],
                                     start=True, stop=False)
                    nc.tensor.matmul(u_ps[:], wuut_b_sb[:], u_t_sb[:],
                                     start=False, stop=True)
                    un = pd.tile([D, B], F32, name="un")
                    nc.scalar.activation(un[:], u_ps[:], LRELU,
                                         bias=b_uu_sb[:], alpha=SLOPE)
                    nc.gpsimd.dma_start(u_out[:], un[:])

    nc.compile()
    return nc


def _prep_inputs(L_t, C_t, U_t, A, A_t,
                 W_Lmsg, b_Lmsg, W_Cmsg, b_Cmsg,
                 W_Lu, b_Lu, W_Cu, b_Cu, W_Uu, b_Uu, msg_bias):
    f16, f32 = np.float16, np.float32

    lp = np.stack([
        np.concatenate([L_t[0], L_t[1]], axis=0),
        np.concatenate([L_t[2], L_t[3]], axis=0),
    ]).astype(f16)                                           # [2,128,NL]
    w2 = np.kron(np.eye(2, dtype=f32), W_Lmsg.T).astype(f16)  # [128,128]
    wcut_od = W_Cu.T.astype(f16)
    wcut_ev = np.concatenate([W_Cu.T[64:128], W_Cu.T[0:64]]).astype(f16)
    wlt = W_Lu.T.astype(f16)
    wlut_a_od = wlt[0:128].copy()
    wlut_a_ev = np.concatenate([wlt[64:128], wlt[0:64]]).astype(f16)
    wlut_b = wlt[128:192].copy()
    wut = W_Uu.T.astype(f32)
    rep = dict(
        lpair=lp, w2lmsg=w2,
        wcut_ev=wcut_ev, wcut_od=wcut_od, wcmsgt=W_Cmsg.T.astype(f16).copy(),
        wlut_a_ev=wlut_a_ev, wlut_a_od=wlut_a_od, wlut_b=wlut_b,
        wuut_a=wut[0:128].copy(), wuut_b=wut[128:192].copy(),
        b_cu=b_Cu.reshape(D, 1).astype(f32),
        b_lu=b_Lu.reshape(D, 1).astype(f32),
        b_uu=b_Uu.reshape(D, 1).astype(f32),
        u_t=U_t.reshape(B, D).T.astype(f32).copy(),
    )
    if msg_bias:
        rep["ones128"] = np.ones((128, 1), f16)
        rep["b_lmsg_row"] = b_Lmsg.reshape(1, D).astype(f32)
        rep["b_cmsg_row"] = b_Cmsg.reshape(1, D).astype(f32)

    At16 = A_t.astype(f16)
    # permute A rows into phase-C arrival order: (half, src_rank, chunk)
    hh, ss, ww, pp = np.meshgrid(np.arange(2), np.arange(NCORES),
                                 np.arange(8), np.arange(128), indexing="ij")
    row_order = (ss * CSH + hh * 1024 + ww * 128 + pp).reshape(-1)
    A16p = A.astype(f16)[row_order]
    in_maps = []
    for r in range(NCORES):
        atr = At16[:, r * CSH:(r + 1) * CSH]                 # [8192, 2048]
        at_tiles = (atr.reshape(16, 4, 128, 2, 1024)
                    .transpose(3, 0, 2, 1, 4).reshape(2, 16, 128, 4096))
        ar = A16p[:, r * LSH:(r + 1) * LSH]                  # [16384, 1024]
        a_tiles = (ar.reshape(32, 4, 128, 1024)
                   .transpose(0, 2, 1, 3).reshape(32, 128, 4096))
        fl = ((r + NCORES // 2) % NCORES) * LSH
        m = dict(rep)
        m.update(
            at_t=np.ascontiguousarray(at_tiles),
            a_t=np.ascontiguousarray(a_tiles),
            c_shard=np.ascontiguousarray(
                C_t[:, :, r * CSH:(r + 1) * CSH]).astype(f16),
            l_shard=np.ascontiguousarray(
                L_t[:, :, r * LSH:(r + 1) * LSH]).astype(f16),
            l_flip=np.ascontiguousarray(L_t[:, :, fl:fl + LSH]).astype(f16),
        )
        in_maps.append(m)
    return in_maps


def kernel(L_t, C_t, U_t, A, A_t,
           W_Lmsg, b_Lmsg, W_Cmsg, b_Cmsg,
           W_Lu, b_Lu, W_Cu, b_Cu, W_Uu, b_Uu, _trace=False):
    args = [np.asarray(x, dtype=np.float32) for x in
            (L_t, C_t, U_t, A, A_t, W_Lmsg, b_Lmsg, W_Cmsg, b_Cmsg,
             W_Lu, b_Lu, W_Cu, b_Cu, W_Uu, b_Uu)]
    (L_t, C_t, U_t, A, A_t, W_Lmsg, b_Lmsg, W_Cmsg, b_Cmsg,
     W_Lu, b_Lu, W_Cu, b_Cu, W_Uu, b_Uu) = args

    msg_bias = bool(np.any(b_Lmsg) or np.any(b_Cmsg))
    if msg_bias not in _CACHE:
        _CACHE[msg_bias] = _build(msg_bias)
    nc = _CACHE[msg_bias]

    in_maps = _prep_inputs(L_t, C_t, U_t, A, A_t, W_Lmsg, b_Lmsg,
                           W_Cmsg, b_Cmsg, W_Lu, b_Lu, W_Cu, b_Cu,
                           W_Uu, b_Uu, msg_bias)
    res = bass_utils.run_bass_kernel_spmd(
        nc, in_maps, core_ids=list(range(NCORES)), trace=_trace)

    C_new = np.concatenate([res.results[r]["c_out"] for r in range(NCORES)],
                           axis=2)
    L_new = np.concatenate([res.results[r]["l_out"] for r in range(NCORES)],
                           axis=2)
    U_new = res.results[0]["u_out"].T.reshape(B, D, 1).astype(np.float32)
    out = (L_new.astype(np.float32), C_new.astype(np.float32), U_new)
    if _trace:
        return out, res
    return out
